# revision 5
# baseline (speedup 1.0000x reference)
"""Trainium2 Bass kernel for the correlation-map embedding module.

Math (per (b, nf) pair):
  f1d = bilinear_down28(feature_i[b, nf])                  # [C, 28, 28]
  f2sel[c, k] = bilinear sample of feature_j[b, nf] at the K knn grid points
  corr[k, :, :] = relu(sum_c f2sel[c, k] * f1d[c, :, :])   # [K, 28, 28]
  out[k] = corr[k] / sum_hw(exp(corr[k])) * 10

Key restructurings vs the reference:
  - only the K=128 selected query positions of f2 are ever computed (4-tap
    weighted gather: ap_gather on GPSIMD + weighting on DVE + the tap
    reduction folded into 4 accumulating matmuls), not the full 784 grid;
  - bilinear taps are exactly (2k, 2k+1) per output index, so the f1
    downsample is a single fused 4-tap weighted sum at 28x28 resolution:
    7 DVE ops on 784-elem tiles with precomputed product-weight planes;
  - the channel contraction runs on the tensor engine in float32r
    (full PE rate, ~1e-3 relative accuracy wrt fp32);
  - epilogue scaling rides the ScalarE activations: relu(corr)*10 via
    activation scale, exp(relu(corr)) via Exp with scale=0.1, final
    normalize via Copy with per-partition scale = 1/denom.

Sharding: pure data parallel — batch dim (16) split across 8 cores, 2 each.
"""

import numpy as np

# hardcoded problem shapes (grading calls kernel(**inputs) standalone)
B, NF, C, H, W = 16, 3, 128, 56, 56
G = 28
K = 128
NCORES = 8
BPC = B // NCORES  # 2
P = 128

_CACHE = {}


def _axis_coords(n_in):
    # float32 arithmetic to match the jax reference bit-for-bit
    src = np.arange(G, dtype=np.float32) * np.float32((n_in - 1) / (G - 1))
    i0 = np.clip(np.floor(src).astype(np.int32), 0, n_in - 2)
    w = (src - i0.astype(np.float32)).astype(np.float32)
    return i0, w


def _host_consts(knn_inds):
    i0h, wh = _axis_coords(H)
    i0w, ww = _axis_coords(W)
    # the even/odd strided-AP downsample assumes taps are (2k, 2k+1)
    assert np.array_equal(i0h, 2 * np.arange(G)) and np.array_equal(i0w, 2 * np.arange(G))

    # fused 4-tap downsample product-weight planes, each [28*28]
    # tap order (u, t): u = H-axis tap, t = W-axis tap
    ah, bh = (1.0 - wh), wh
    aw, bw = (1.0 - ww), ww
    w4 = np.stack(
        [
            np.outer(ah, aw).reshape(-1),
            np.outer(ah, bw).reshape(-1),
            np.outer(bh, aw).reshape(-1),
            np.outer(bh, bw).reshape(-1),
        ]
    ).astype(np.float32)  # [4, 784]

    # gather indices/weights for the 4 bilinear taps of each knn point
    knn = np.asarray(knn_inds).astype(np.int64)  # [NF, K, 2]
    gidx = np.zeros((NF, P, 32), dtype=np.int16)
    gwts = np.zeros((NF, 4 * K), dtype=np.float32)
    for nf in range(NF):
        h2 = knn[nf, :, 1]
        w2 = knn[nf, :, 0]
        r0 = i0h[h2]
        c0 = i0w[w2]
        # j = k*4 + t ordering: gathered tile is [P, K, 4]
        pos = np.stack(
            [r0 * W + c0, r0 * W + c0 + 1, (r0 + 1) * W + c0, (r0 + 1) * W + c0 + 1],
            axis=1,
        ).reshape(-1)
        wt = np.stack(
            [ah[h2] * aw[w2], ah[h2] * bw[w2], bh[h2] * aw[w2], bh[h2] * bw[w2]],
            axis=1,
        ).reshape(-1)
        gwts[nf] = wt.astype(np.float32)
        # ap_gather index layout: gathered element j comes from partition j%16,
        # slot j//16 of its 16-partition group; replicate across the 8 groups
        wrapped = pos.reshape(32, 16).T.astype(np.int16)  # [16, 32]
        gidx[nf] = np.tile(wrapped, (8, 1))
    return w4, gidx, gwts


def _build_bass():
    import concourse.bacc as bacc
    import concourse.tile as tile
    from concourse import mybir

    f32 = mybir.dt.float32
    f32r = mybir.dt.float32r
    i16 = mybir.dt.int16
    AF = mybir.ActivationFunctionType
    ALU = mybir.AluOpType

    nc = bacc.Bacc()
    fi = nc.dram_tensor("fi", [BPC, NF, C, H, W], f32, kind="ExternalInput")
    fj = nc.dram_tensor("fj", [BPC, NF, C, H, W], f32, kind="ExternalInput")
    w4_d = nc.dram_tensor("w4", [4, G * G], f32, kind="ExternalInput")
    gidx_d = nc.dram_tensor("gidx", [NF, P, 32], i16, kind="ExternalInput")
    gw_d = nc.dram_tensor("gw", [NF, 4 * K], f32, kind="ExternalInput")
    out_d = nc.dram_tensor("out", [BPC, NF, K, G, G], f32, kind="ExternalOutput")

    with tile.TileContext(nc) as tc:
        with (
            tc.tile_pool(name="consts", bufs=1) as consts,
            tc.tile_pool(name="feat", bufs=3) as feat,
            tc.tile_pool(name="work", bufs=2) as work,
            tc.tile_pool(name="psum", bufs=3, space="PSUM") as pspool,
            tc.tile_pool(name="outp", bufs=3) as outp,
        ):
            w4_t = []
            for u in range(4):
                t = consts.tile([P, G * G], f32, tag=f"w4_{u}")
                nc.sync.dma_start(out=t, in_=w4_d[u : u + 1].to_broadcast([P, G * G]))
                w4_t.append(t)
            gidx_t = []
            gw_t = []
            for nf in range(NF):
                it = consts.tile([P, 32], i16, tag=f"gidx{nf}")
                nc.sync.dma_start(out=it, in_=gidx_d[nf])
                gidx_t.append(it)
                wt = consts.tile([P, 4 * K], f32, tag=f"gw{nf}")
                nc.sync.dma_start(out=wt, in_=gw_d[nf : nf + 1].to_broadcast([P, 4 * K]))
                gw_t.append(wt)

            for b in range(BPC):
                for nf in range(NF):
                    f1 = feat.tile([P, H, W], f32, tag="f1")
                    nc.sync.dma_start(out=f1, in_=fi[b, nf])
                    f2 = feat.tile([P, H, W], f32, tag="f2")
                    nc.sync.dma_start(out=f2, in_=fj[b, nf])

                    # f2 at the K selected grid points: gather the 4 bilinear
                    # taps (GPSIMD), apply tap weights (DVE)
                    g = work.tile([P, K, 4], f32, tag="g")
                    nc.gpsimd.ap_gather(
                        g.rearrange("p k t -> p (k t)"),
                        f2.rearrange("p h w -> p (h w)"),
                        gidx_t[nf],
                        channels=P,
                        num_elems=H * W,
                        d=1,
                        num_idxs=4 * K,
                    )
                    gg = work.tile([P, K, 4], f32r, tag="gg")
                    nc.vector.tensor_mul(
                        gg.rearrange("p k t -> p (k t)"),
                        g.rearrange("p k t -> p (k t)"),
                        gw_t[nf],
                    )

                    # f1 downsample: fused 4-tap weighted sum at 28x28 (DVE)
                    f1v = f1.rearrange(
                        "p (h uu) (w tt) -> p h uu w tt", uu=2, tt=2
                    )
                    m = []
                    for u in range(2):
                        for t in range(2):
                            mt = work.tile([P, G, G], f32, tag=f"m{u}{t}")
                            nc.vector.tensor_mul(
                                mt,
                                f1v[:, :, u, :, t],
                                w4_t[2 * u + t].rearrange("p (h g) -> p h g", g=G),
                            )
                            m.append(mt)
                    a0 = work.tile([P, G, G], f32, tag="a0")
                    nc.vector.tensor_add(a0, m[0], m[1])
                    a1 = work.tile([P, G, G], f32, tag="a1")
                    nc.vector.tensor_add(a1, m[2], m[3])
                    f1d = work.tile([P, G, G], f32r, tag="f1d")
                    nc.vector.tensor_add(f1d, a0, a1)

                    # correlation: corr[k, q] = sum_c sum_t gg[c,k,t] * f1d[c,q]
                    # tap reduction folded into 4 accumulating matmuls
                    ps = pspool.tile([P, G * G], f32, tag="ps")
                    rhs = f1d.rearrange("p h g -> p (h g)")
                    # matmul outputs must not cross a PSUM bank (512 f32);
                    # 512+272 split, both N >= 256 for full float32r rate
                    half = 512
                    for t in range(4):
                        lhsT = gg[:, :, t]
                        nc.tensor.matmul(
                            ps[:, :half],
                            lhsT=lhsT,
                            rhs=rhs[:, :half],
                            start=(t == 0),
                            stop=(t == 3),
                        )
                        nc.tensor.matmul(
                            ps[:, half:],
                            lhsT=lhsT,
                            rhs=rhs[:, half:],
                            start=(t == 0),
                            stop=(t == 3),
                        )

                    # epilogue on ScalarE: r = 10*relu(corr); s = sum(exp(r/10));
                    # out = r * (1/s)
                    r = outp.tile([P, G * G], f32, tag="r")
                    nc.scalar.activation(r, ps, AF.Relu, scale=10.0)
                    e = work.tile([P, G * G], f32, tag="e")
                    s = work.tile([P, 1], f32, tag="s")
                    nc.scalar.activation(e, r, AF.Exp, scale=0.1, accum_out=s)
                    rec = work.tile([P, 1], f32, tag="rec")
                    nc.vector.reciprocal(rec, s)
                    o = outp.tile([P, G * G], f32, tag="o")
                    nc.scalar.activation(o, r, AF.Copy, scale=rec)
                    nc.sync.dma_start(
                        out=out_d[b, nf].rearrange("k g1 g2 -> k (g1 g2)"), in_=o
                    )
    return nc


def _get_bass():
    if "nc" not in _CACHE:
        nc = _build_bass()
        # run the Bacc passes (reg alloc, library-load insertion) before the
        # PJRT path serializes the module
        if not nc.is_finalized():
            nc.finalize()
        _CACHE["nc"] = nc
    return _CACHE["nc"]


def kernel(feature_i, feature_j, mask, optical_flow, knn_inds):
    from concourse import bass_utils

    nc = _get_bass()
    w4, gidx, gwts = _host_consts(knn_inds)

    fi = np.ascontiguousarray(np.asarray(feature_i, dtype=np.float32))
    fj = np.ascontiguousarray(np.asarray(feature_j, dtype=np.float32))

    in_maps = []
    for core in range(NCORES):
        lo = core * BPC
        in_maps.append(
            {
                "fi": fi[lo : lo + BPC],
                "fj": fj[lo : lo + BPC],
                "w4": w4,
                "gidx": gidx,
                "gw": gwts,
            }
        )

    res = bass_utils.run_bass_kernel_spmd(nc, in_maps, core_ids=list(range(NCORES)))
    out = np.concatenate([res.results[c]["out"] for c in range(NCORES)], axis=0)
    return out.astype(np.float32)


# revision 7
# speedup vs baseline: 854.3696x; 854.3696x over previous
"""Trainium2 Bass kernel for the correlation-map embedding module.

Math (per (b, nf) pair):
  f1d = bilinear_down28(feature_i[b, nf])                  # [C, 28, 28]
  f2sel[c, k] = bilinear sample of feature_j[b, nf] at the K knn grid points
  corr[k, :, :] = relu(sum_c f2sel[c, k] * f1d[c, :, :])   # [K, 28, 28]
  out[k] = corr[k] / sum_hw(exp(corr[k])) * 10

Key restructurings vs the reference:
  - only the K=128 selected query positions of f2 are ever computed (4-tap
    weighted gather: ap_gather on GPSIMD + weighting on DVE + the tap
    reduction folded into 4 accumulating matmuls), not the full 784 grid;
  - bilinear taps are exactly (2k, 2k+1) per output index, so the f1
    downsample is a single fused 4-tap weighted sum at 28x28 resolution:
    7 DVE ops on 784-elem tiles with precomputed product-weight planes;
  - the channel contraction runs on the tensor engine in float32r
    (full PE rate, ~1e-3 relative accuracy wrt fp32);
  - epilogue scaling rides the ScalarE activations: relu(corr)*10 via
    activation scale, exp(relu(corr)) via Exp with scale=0.1, final
    normalize via Copy with per-partition scale = 1/denom.

Sharding: pure data parallel — batch dim (16) split across 8 cores, 2 each.
"""

import numpy as np

# hardcoded problem shapes (grading calls kernel(**inputs) standalone)
B, NF, C, H, W = 16, 3, 128, 56, 56
G = 28
K = 128
NCORES = 8
BPC = B // NCORES  # 2
P = 128

_CACHE = {}


def _axis_coords(n_in):
    # float32 arithmetic to match the jax reference bit-for-bit
    src = np.arange(G, dtype=np.float32) * np.float32((n_in - 1) / (G - 1))
    i0 = np.clip(np.floor(src).astype(np.int32), 0, n_in - 2)
    w = (src - i0.astype(np.float32)).astype(np.float32)
    return i0, w


def _host_consts(knn_inds):
    i0h, wh = _axis_coords(H)
    i0w, ww = _axis_coords(W)
    # the even/odd strided-AP downsample assumes taps are (2k, 2k+1)
    assert np.array_equal(i0h, 2 * np.arange(G)) and np.array_equal(i0w, 2 * np.arange(G))

    # fused 4-tap downsample product-weight planes, each [28*28]
    # tap order (u, t): u = H-axis tap, t = W-axis tap
    ah, bh = (1.0 - wh), wh
    aw, bw = (1.0 - ww), ww
    w4 = np.stack(
        [
            np.outer(ah, aw).reshape(-1),
            np.outer(ah, bw).reshape(-1),
            np.outer(bh, aw).reshape(-1),
            np.outer(bh, bw).reshape(-1),
        ]
    ).astype(np.float32)  # [4, 784]

    # gather indices/weights for the 4 bilinear taps of each knn point
    knn = np.asarray(knn_inds).astype(np.int64)  # [NF, K, 2]
    gidx = np.zeros((NF, P, 32), dtype=np.int16)
    gwts = np.zeros((NF, 4 * K), dtype=np.float32)
    for nf in range(NF):
        h2 = knn[nf, :, 1]
        w2 = knn[nf, :, 0]
        r0 = i0h[h2]
        c0 = i0w[w2]
        # j = k*4 + t ordering: gathered tile is [P, K, 4]
        pos = np.stack(
            [r0 * W + c0, r0 * W + c0 + 1, (r0 + 1) * W + c0, (r0 + 1) * W + c0 + 1],
            axis=1,
        ).reshape(-1)
        wt = np.stack(
            [ah[h2] * aw[w2], ah[h2] * bw[w2], bh[h2] * aw[w2], bh[h2] * bw[w2]],
            axis=1,
        ).reshape(-1)
        gwts[nf] = wt.astype(np.float32)
        # ap_gather index layout: gathered element j comes from partition j%16,
        # slot j//16 of its 16-partition group; replicate across the 8 groups
        wrapped = pos.reshape(32, 16).T.astype(np.int16)  # [16, 32]
        gidx[nf] = np.tile(wrapped, (8, 1))
    return w4, gidx, gwts


def _build_bass(repeat=1, mode="full"):
    """mode: "full" = real kernel; "dma" = only the DMA traffic (roofline probe).
    repeat: clone the whole per-pair pipeline R times (idempotent) so HW time
    can be measured by differencing two R values."""
    import concourse.bacc as bacc
    import concourse.tile as tile
    from concourse import mybir

    f32 = mybir.dt.float32
    f32r = mybir.dt.float32r
    i16 = mybir.dt.int16
    AF = mybir.ActivationFunctionType
    ALU = mybir.AluOpType

    nc = bacc.Bacc()
    fi = nc.dram_tensor("fi", [BPC, NF, C, H, W], f32, kind="ExternalInput")
    fj = nc.dram_tensor("fj", [BPC, NF, C, H, W], f32, kind="ExternalInput")
    w4_d = nc.dram_tensor("w4", [4, G * G], f32, kind="ExternalInput")
    gidx_d = nc.dram_tensor("gidx", [NF, P, 32], i16, kind="ExternalInput")
    gw_d = nc.dram_tensor("gw", [NF, 4 * K], f32, kind="ExternalInput")
    out_d = nc.dram_tensor("out", [BPC, NF, K, G, G], f32, kind="ExternalOutput")

    with tile.TileContext(nc) as tc:
        with (
            tc.tile_pool(name="consts", bufs=1) as consts,
            tc.tile_pool(name="feat", bufs=3) as feat,
            tc.tile_pool(name="work", bufs=2) as work,
            tc.tile_pool(name="psum", bufs=3, space="PSUM") as pspool,
            tc.tile_pool(name="outp", bufs=3) as outp,
        ):
            w4_t = []
            for u in range(4):
                t = consts.tile([P, G * G], f32, tag=f"w4_{u}")
                nc.sync.dma_start(out=t, in_=w4_d[u : u + 1].to_broadcast([P, G * G]))
                w4_t.append(t)
            gidx_t = []
            gw_t = []
            for nf in range(NF):
                it = consts.tile([P, 32], i16, tag=f"gidx{nf}")
                nc.sync.dma_start(out=it, in_=gidx_d[nf])
                gidx_t.append(it)
                wt = consts.tile([P, 4 * K], f32, tag=f"gw{nf}")
                nc.sync.dma_start(out=wt, in_=gw_d[nf : nf + 1].to_broadcast([P, 4 * K]))
                gw_t.append(wt)

            for rep in range(repeat):
              for b in range(BPC):
                for nf in range(NF):
                    f1 = feat.tile([P, H, W], f32, tag="f1")
                    nc.sync.dma_start(out=f1, in_=fi[b, nf])
                    f2 = feat.tile([P, H, W], f32, tag="f2")
                    nc.sync.dma_start(out=f2, in_=fj[b, nf])

                    if mode == "dma":
                        o = outp.tile([P, G * G], f32, tag="o")
                        nc.vector.memset(o[:1, :1], 0.0)
                        nc.sync.dma_start(
                            out=out_d[b, nf].rearrange("k g1 g2 -> k (g1 g2)"), in_=o
                        )
                        continue

                    # f2 at the K selected grid points: gather the 4 bilinear
                    # taps (GPSIMD), apply tap weights (DVE)
                    g = work.tile([P, K, 4], f32, tag="g")
                    nc.gpsimd.ap_gather(
                        g.rearrange("p k t -> p (k t)"),
                        f2.rearrange("p h w -> p (h w)"),
                        gidx_t[nf],
                        channels=P,
                        num_elems=H * W,
                        d=1,
                        num_idxs=4 * K,
                    )
                    gg = work.tile([P, K, 4], f32r, tag="gg")
                    nc.vector.tensor_mul(
                        gg.rearrange("p k t -> p (k t)"),
                        g.rearrange("p k t -> p (k t)"),
                        gw_t[nf],
                    )

                    # f1 downsample: fused 4-tap weighted sum at 28x28 (DVE)
                    f1v = f1.rearrange(
                        "p (h uu) (w tt) -> p h uu w tt", uu=2, tt=2
                    )
                    m = []
                    for u in range(2):
                        for t in range(2):
                            mt = work.tile([P, G, G], f32, tag=f"m{u}{t}")
                            nc.vector.tensor_mul(
                                mt,
                                f1v[:, :, u, :, t],
                                w4_t[2 * u + t].rearrange("p (h g) -> p h g", g=G),
                            )
                            m.append(mt)
                    a0 = work.tile([P, G, G], f32, tag="a0")
                    nc.vector.tensor_add(a0, m[0], m[1])
                    a1 = work.tile([P, G, G], f32, tag="a1")
                    nc.vector.tensor_add(a1, m[2], m[3])
                    f1d = work.tile([P, G, G], f32r, tag="f1d")
                    nc.vector.tensor_add(f1d, a0, a1)

                    # correlation: corr[k, q] = sum_c sum_t gg[c,k,t] * f1d[c,q]
                    # tap reduction folded into 4 accumulating matmuls
                    ps = pspool.tile([P, G * G], f32, tag="ps")
                    rhs = f1d.rearrange("p h g -> p (h g)")
                    # matmul outputs must not cross a PSUM bank (512 f32);
                    # 512+272 split, both N >= 256 for full float32r rate
                    half = 512
                    for t in range(4):
                        lhsT = gg[:, :, t]
                        nc.tensor.matmul(
                            ps[:, :half],
                            lhsT=lhsT,
                            rhs=rhs[:, :half],
                            start=(t == 0),
                            stop=(t == 3),
                        )
                        nc.tensor.matmul(
                            ps[:, half:],
                            lhsT=lhsT,
                            rhs=rhs[:, half:],
                            start=(t == 0),
                            stop=(t == 3),
                        )

                    # epilogue on ScalarE: r = 10*relu(corr); s = sum(exp(r/10));
                    # out = r * (1/s)
                    r = outp.tile([P, G * G], f32, tag="r")
                    nc.scalar.activation(r, ps, AF.Relu, scale=10.0)
                    e = work.tile([P, G * G], f32, tag="e")
                    s = work.tile([P, 1], f32, tag="s")
                    nc.scalar.activation(e, r, AF.Exp, scale=0.1, accum_out=s)
                    rec = work.tile([P, 1], f32, tag="rec")
                    nc.vector.reciprocal(rec, s)
                    o = outp.tile([P, G * G], f32, tag="o")
                    nc.scalar.activation(o, r, AF.Copy, scale=rec)
                    nc.sync.dma_start(
                        out=out_d[b, nf].rearrange("k g1 g2 -> k (g1 g2)"), in_=o
                    )
    return nc


def _get_bass():
    if "nc" not in _CACHE:
        nc = _build_bass()
        # run the Bacc passes (reg alloc, library-load insertion) before the
        # PJRT path serializes the module
        if not nc.is_finalized():
            nc.finalize()
        _CACHE["nc"] = nc
    return _CACHE["nc"]


def kernel(feature_i, feature_j, mask, optical_flow, knn_inds):
    from concourse import bass_utils

    nc = _get_bass()
    w4, gidx, gwts = _host_consts(knn_inds)

    fi = np.ascontiguousarray(np.asarray(feature_i, dtype=np.float32))
    fj = np.ascontiguousarray(np.asarray(feature_j, dtype=np.float32))

    in_maps = []
    for core in range(NCORES):
        lo = core * BPC
        in_maps.append(
            {
                "fi": fi[lo : lo + BPC],
                "fj": fj[lo : lo + BPC],
                "w4": w4,
                "gidx": gidx,
                "gw": gwts,
            }
        )

    res = bass_utils.run_bass_kernel_spmd(nc, in_maps, core_ids=list(range(NCORES)))
    out = np.concatenate([res.results[c]["out"] for c in range(NCORES)], axis=0)
    return out.astype(np.float32)
